# revision 51
# baseline (speedup 1.0000x reference)
"""Trainium2 Bass kernel for nn_Attention (B=8, L=2048, D=512).

Strategy: data-parallel over batch — one batch element per NeuronCore
(8 cores). The host feeds each core its batch slice transposed and
pre-cast to bf16, plus weight-only precomputations (layout/weight
prep; all activation arithmetic runs on device):
  - softmax is shift-invariant, so q.k = (x wq^T + bq).(s wk^T + bk)
    reduces to x A s^T + sw[k] with A = wq^T wk and sw = s.(bq wk)/sqrt(D)
    — the query-constant terms drop. This removes one of the two score
    projections entirely.
Per core:
  - x^T/s^T arrive pre-cast to bf16 in matmul-ready SBUF layouts via
    direct DMA — no on-device casts.  DMA pieces are sized/ordered so
    descriptor rows are large (4KB, 64B-aligned) and land in the V
    phase's consumption order on one HW queue
  - junk warm-up matmuls on memset constants run during the initial
    DMA window so the PE HAM clock-gate is already at 8/8 when real
    matmuls start
  - V projection: moving operand is [wvec | wv^T] (513 cols, split
    257+256 across two PSUM banks), so the per-key score bias
    sw = s.wvec falls out of column 0 of the same matmul group
  - T^T = A^T x^T, one 512-column block per q-block round, interleaved
    with the attention phases so each x block's DMA deadline is a full
    round later
  - scores^T = s^T-stationary x T^T-moving  => [k, q] layout, so the
    softmax key-dim lands on partitions
  - E = exp(scale * scores^T + sw[k]) on ScalarE, sw as the
    per-partition activation bias (no max-subtraction needed:
    shift-invariance again, and scores are O(1) here)
  - context: moving operand is [1 | V] (513 cols, split 257+256), so
    the key-dim softmax denominator is column 0 of the same PSUM
    accumulation — no transposes, row-sum matmuls or DVE adds
  - out = ctx * recip(sum) + bv via fused DVE scalar_tensor_tensor
    (v-bias folds out of the projection: softmax rows sum to 1), in
    two 256-column halves so the output DMA overlaps the second half
All matmuls run in bf16 with fp32 PSUM accumulation.

The mask input is all-ones per the problem spec; kernel() verifies that
on the host and falls back to an exact numpy implementation for any
other mask. A per-batch spot-check guards the device path (retry, then
exact-host fallback) so out-of-spec inputs or a bad run can never
return wrong results.
"""

import numpy as np

B, L, D = 8, 2048, 512
P = 128
LT = L // P  # 16 l-tiles
DC = D // P  # 4 d/e chunks
NQ = 512  # q-block width
QB = L // NQ  # 4 q blocks
NB = L // NQ  # 4 l-blocks (512 rows each)
N_CORES = 8
SCALE = 1.0 / float(np.sqrt(D))
N_WARM = 48  # junk warm-up matmuls (N=128 each) during the DMA head

_cache = {}


def _build_fast():
    import concourse.tile as tile
    from concourse import bacc, mybir
    from concourse.bass import ds

    F32 = mybir.dt.float32
    BF16 = mybir.dt.bfloat16
    AF = mybir.ActivationFunctionType

    nc = bacc.Bacc(
        "TRN2", target_bir_lowering=False, debug=False, num_devices=N_CORES
    )
    # all big tensors arrive pre-transposed/pre-cast in SBUF layout,
    # arranged for large contiguous DMA descriptor rows (4 KB where
    # possible — small rows run the DGE at ~70 GB/s instead of ~300)
    xT_ext = nc.dram_tensor(
        "inputT", [P, NB, DC, NQ], BF16, kind="ExternalInput"
    )
    # head = [wvA | pad | s-tile0 | s-tile1]: wvA = [wvec | wv^T] per
    # chunk (col 0 = (bq @ wk) * scale), padded to a 64B-aligned row
    # size — misaligned DMA rows run at roughly half bandwidth
    NWVA_PAD = DC * (D + 1) + 28  # 2080 cols = 4160 B, 64-aligned
    NHEAD = NWVA_PAD + 2 * D
    head_ext = nc.dram_tensor("headW", [P, NHEAD], BF16, kind="ExternalInput")
    # states tiles 2-15 partition-major: 4 KB rows per piece
    sb_ext = nc.dram_tensor(
        "statesB", [P, LT - 2, DC * P], BF16, kind="ExternalInput"
    )
    # amat = wq.T @ wk (scores reduce to x @ amat @ s.T plus a per-key
    # bias; the query-constant terms drop out of the softmax)
    amat_ext = nc.dram_tensor("amat", [P, DC * D], BF16, kind="ExternalInput")
    bv_ext = nc.dram_tensor("bv", [D], F32, kind="ExternalInput")
    out_ext = nc.dram_tensor("out", [L, D], F32, kind="ExternalOutput")

    with tile.TileContext(nc) as tc:
        with (
            tc.tile_pool(name="consts", bufs=1) as consts,
            tc.tile_pool(name="persist", bufs=1) as persist,
            tc.tile_pool(name="et", bufs=2) as et_pool,
            tc.tile_pool(name="outp", bufs=3) as outp,
            tc.tile_pool(name="psum_mm", bufs=4, space="PSUM") as psum_mm,
            tc.tile_pool(name="psum_ua", bufs=2, space="PSUM") as psum_ua,
            tc.tile_pool(name="psum_ub", bufs=2, space="PSUM") as psum_ub,
        ):
            # constants (no DMA deps — memsets run right after preamble)
            warm_src = consts.tile([P, P], BF16, tag="warm_src")
            nc.gpsimd.memset(warm_src[:], 1.0)
            ones_st = consts.tile([1, P], BF16, tag="ones_st")
            nc.gpsimd.memset(ones_st[:], 1.0)

            # persistent bf16 tensors
            xT = persist.tile([P, NB, DC, NQ], BF16, tag="xT")
            headw = persist.tile([P, NHEAD], BF16, tag="headw")
            sT = persist.tile([P, LT - 2, DC, P], BF16, tag="sT")
            TT = persist.tile([P, DC, L], BF16, tag="TT")

            # stationary s-tile / wvA slices (head tiles live in headw)
            def s_stat(t, c):
                if t < 2:
                    return headw[:, ds(NWVA_PAD + (t * DC + c) * P, P)]
                return sT[:, t - 2, c, :]

            def wvA_sl(c, off, n):
                return headw[:, ds(c * (D + 1) + off, n)]
            # V_aug = [1 | V]: col 0 is all-ones so the context matmul's
            # first PSUM column accumulates the softmax denominator
            V_aug = persist.tile([P, LT, D + 1], BF16, tag="V_aug")
            nc.gpsimd.memset(V_aug[:, :, ds(0, 1)], 1.0)
            amat = persist.tile([P, DC, D], BF16, tag="amat")
            sw_sb = persist.tile([P, LT], F32, tag="sw_sb")

            # ---- warm-up matmuls: keep the PE busy through the DMA
            # head so HAM un-throttles before real work arrives ----
            # rides a context-pool bank: the first context matmul is
            # ~30us after the last warm-up, so the WAW dep is free
            warm_ps = psum_ua.tile([P, NQ], F32, tag="ps_ua")
            for _ in range(N_WARM):
                nc.tensor.matmul(
                    warm_ps[:, ds(0, P)],
                    warm_src[:],
                    warm_src[:],
                    start=True,
                    stop=True,
                )

            # ---- DMA issue order = landing order (one FIFO): the
            # combined head first, then s pieces at the V phase's
            # consumption cadence, then weights and x blocks ----
            nc.sync.dma_start(
                headw[:, ds(0, NWVA_PAD)], head_ext.ap()[:, ds(0, NWVA_PAD)]
            )
            nc.sync.dma_start(
                headw[:, ds(NWVA_PAD, 2 * D)],
                head_ext.ap()[:, ds(NWVA_PAD, 2 * D)],
            )
            sb_ap = sb_ext.ap().rearrange("p t (c j) -> p t c j", c=DC)
            for r0, nt in ((0, 2), (2, 2), (4, 4), (8, 4), (12, 2)):
                nc.sync.dma_start(
                    sT[:, ds(r0, nt), :, :], sb_ap[:, ds(r0, nt), :, :]
                )
            bv_f32 = consts.tile([1, D], F32, tag="bv_f32")
            nc.sync.dma_start(
                bv_f32[:], bv_ext.ap().rearrange("(one d) -> one d", one=1)
            )
            nc.sync.dma_start(
                amat[:], amat_ext.ap().rearrange("p (c e) -> p c e", c=DC)
            )
            for lb in range(NB):
                nc.sync.dma_start(
                    xT[:, ds(lb, 1), :, :], xT_ext.ap()[:, ds(lb, 1), :, :]
                )

            # ---- Phase A: V projection (+ sw in column 0) ----
            for t in range(LT):
                psA = psum_mm.tile([P, NQ], F32, tag="ps_mm")
                psB = psum_mm.tile([P, NQ], F32, tag="ps_mm")
                for c in range(DC):
                    nc.tensor.matmul(
                        psA[:, ds(0, 257)],
                        s_stat(t, c),
                        wvA_sl(c, 0, 257),
                        start=(c == 0),
                        stop=(c == DC - 1),
                    )
                    nc.tensor.matmul(
                        psB[:, ds(0, 256)],
                        s_stat(t, c),
                        wvA_sl(c, 257, 256),
                        start=(c == 0),
                        stop=(c == DC - 1),
                    )
                nc.vector.tensor_copy(sw_sb[:, ds(t, 1)], psA[:, ds(0, 1)])
                nc.vector.tensor_copy(
                    V_aug[:, t, ds(1, 256)], psA[:, ds(1, 256)]
                )
                nc.vector.tensor_copy(
                    V_aug[:, t, ds(257, 256)], psB[:, ds(0, 256)]
                )

            # BV: bv broadcast to all 128 partitions (ones-column matmul);
            # rides the PE stream between phases, consumed only at the end
            bv_bf = consts.tile([1, D], BF16, tag="bv_bf")
            nc.vector.tensor_copy(bv_bf[:], bv_f32[:])
            bv_ps = psum_mm.tile([P, D], F32, tag="ps_mm")
            nc.tensor.matmul(
                bv_ps[:], ones_st[:, :], bv_bf[:, :], start=True, stop=True
            )
            BV = consts.tile([P, D], F32, tag="BV")
            nc.vector.tensor_copy(BV[:], bv_ps[:])

            # ---- Phases B+C interleaved per q-block: project T for
            # block qb (x block qb only), then scores + context.  This
            # defers each x block's deadline by a full qb round, so the
            # DMA link can prioritize the V phase's s tiles. ----
            for qb in range(QB):
                lb = qb
                for e in range(DC):
                    ps = psum_mm.tile([P, NQ], F32, tag="ps_mm")
                    for c in range(DC):
                        nc.tensor.matmul(
                            ps[:],
                            amat[:, c, ds(e * P, P)],
                            xT[:, lb, c, :],
                            start=(c == 0),
                            stop=(c == DC - 1),
                        )
                    nc.scalar.copy(TT[:, e, ds(lb * NQ, NQ)], ps[:])

                ET = et_pool.tile([P, LT, NQ], BF16, tag="ET")
                for kt in range(LT):
                    ps = psum_mm.tile([P, NQ], F32, tag="ps_mm")
                    for e in range(DC):
                        nc.tensor.matmul(
                            ps[:],
                            s_stat(kt, e),
                            TT[:, e, ds(qb * NQ, NQ)],
                            start=(e == 0),
                            stop=(e == DC - 1),
                        )
                    nc.scalar.activation(
                        ET[:, kt, :],
                        ps[:],
                        AF.Exp,
                        bias=sw_sb[:, ds(kt, 1)],
                        scale=SCALE,
                    )

                for j in range(NQ // P):
                    uA = psum_ua.tile([P, NQ], F32, tag="ps_ua")
                    uB = psum_ub.tile([P, NQ], F32, tag="ps_ub")
                    for kt in range(LT):
                        nc.tensor.matmul(
                            uA[:, ds(0, 257)],
                            ET[:, kt, ds(j * P, P)],
                            V_aug[:, kt, ds(0, 257)],
                            start=(kt == 0),
                            stop=(kt == LT - 1),
                        )
                        nc.tensor.matmul(
                            uB[:, ds(0, 256)],
                            ET[:, kt, ds(j * P, P)],
                            V_aug[:, kt, ds(257, 256)],
                            start=(kt == 0),
                            stop=(kt == LT - 1),
                        )
                    rec = outp.tile([P, 1], F32, tag="rec")
                    nc.vector.reciprocal(rec[:], uA[:, ds(0, 1)])
                    o = outp.tile([P, D], F32, tag="o")
                    row = (qb * (NQ // P) + j) * P
                    nc.vector.scalar_tensor_tensor(
                        o[:, ds(0, 256)],
                        uA[:, ds(1, 256)],
                        rec[:],
                        BV[:, ds(0, 256)],
                        op0=mybir.AluOpType.mult,
                        op1=mybir.AluOpType.add,
                    )
                    nc.sync.dma_start(
                        out_ext.ap()[ds(row, P), ds(0, 256)], o[:, ds(0, 256)]
                    )
                    nc.vector.scalar_tensor_tensor(
                        o[:, ds(256, 256)],
                        uB[:, ds(0, 256)],
                        rec[:],
                        BV[:, ds(256, 256)],
                        op0=mybir.AluOpType.mult,
                        op1=mybir.AluOpType.add,
                    )
                    # final tile: issue the B-half store from the scalar
                    # queue so the two issues overlap on the tail path
                    last = qb == QB - 1 and j == NQ // P - 1
                    eng_o = nc.scalar if last else nc.sync
                    eng_o.dma_start(
                        out_ext.ap()[ds(row, P), ds(256, 256)],
                        o[:, ds(256, 256)],
                    )

    nc.compile()
    return nc


def _make_in_maps(input, states, wq, bq, wk, bk, wv, bv):
    import ml_dtypes

    BF = ml_dtypes.bfloat16
    wq64 = np.asarray(wq, dtype=np.float64)
    wk64 = np.asarray(wk, dtype=np.float64)
    amat = (wq64.T @ wk64).astype(np.float32)  # [D, D]
    wvec = ((np.asarray(bq, dtype=np.float64) @ wk64) * SCALE).astype(
        np.float32
    )  # [D]
    wvT = np.asarray(wv, dtype=np.float32).T  # [D, D]
    wvA = np.concatenate([wvec[:, None], wvT], axis=1)  # [D, D+1]
    amat_h = np.ascontiguousarray(
        amat.reshape(DC, P, D).transpose(1, 0, 2).reshape(P, DC * D).astype(BF)
    )
    wvA_h = wvA.reshape(DC, P, D + 1).transpose(1, 0, 2).astype(BF)  # [p,c,e]
    bv = np.ascontiguousarray(bv, dtype=np.float32)
    in_maps = []
    for i in range(N_CORES):
        xt = np.asarray(input[i], dtype=np.float32).T.astype(BF)  # [D, L]
        st = np.asarray(states[i], dtype=np.float32).T.astype(BF)
        # x: [p, block, chunk, col-in-block]
        x_h = np.ascontiguousarray(
            xt.reshape(DC, P, NB, NQ).transpose(1, 2, 0, 3)
        )
        # s tile-major [t, p, c, j]
        s_t = st.reshape(DC, P, LT, P).transpose(2, 1, 0, 3)
        # head = [wvA | pad-to-64B | s-tile0 | s-tile1] per partition row
        head = np.concatenate(
            [
                wvA_h.reshape(P, DC * (D + 1)),
                np.zeros((P, 28), dtype=BF),
                s_t[0].reshape(P, D),
                s_t[1].reshape(P, D),
            ],
            axis=1,
        )
        s_blk = np.ascontiguousarray(
            s_t[2:].transpose(1, 0, 2, 3).reshape(P, LT - 2, DC * P)
        )
        in_maps.append(
            {
                "inputT": x_h,
                "headW": np.ascontiguousarray(head),
                "statesB": s_blk,
                "amat": amat_h,
                "bv": bv,
            }
        )
    return in_maps


def _spot_check(out, input, states, wq, bq, wk, bk, wv, bv):
    """Recompute a few query rows per batch on host; True iff they match."""
    rows = [37, 911, 1500, 2047]
    for i in range(N_CORES):
        k = states[i].astype(np.float64) @ wk.T.astype(np.float64) + bk
        v = states[i].astype(np.float64) @ wv.T.astype(np.float64) + bv
        for r in rows:
            q = input[i, r].astype(np.float64) @ wq.T.astype(np.float64) + bq
            s = (k @ q) / np.sqrt(float(D))
            s -= s.max()
            e = np.exp(s)
            ref_row = (e @ v) / e.sum()
            got = out[i, r].astype(np.float64)
            err = np.linalg.norm(got - ref_row) / max(
                np.linalg.norm(ref_row), 1e-30
            )
            if not np.isfinite(err) or err > 0.05:
                return False
    return True


def _run_fast(input, states, wq, bq, wk, bk, wv, bv):
    from concourse.bass_utils import run_bass_kernel_spmd

    if "fast" not in _cache:
        _cache["fast"] = _build_fast()
    nc = _cache["fast"]
    in_maps = _make_in_maps(input, states, wq, bq, wk, bk, wv, bv)
    for _attempt in range(2):
        res = run_bass_kernel_spmd(nc, in_maps, core_ids=list(range(N_CORES)))
        out = np.stack(
            [res.results[i]["out"] for i in range(N_CORES)], axis=0
        )
        if _spot_check(out, input, states, wq, bq, wk, bk, wv, bv):
            return out
    # two bad device runs in a row: fall back to the exact host path
    ones = np.ones((B, L, L), dtype=np.int32)
    return _numpy_ref(input, states, ones, wq, bq, wk, bk, wv, bv)


def _numpy_ref(input, states, mask, wq, bq, wk, bk, wv, bv):
    # exact fallback for non-all-ones masks (never taken for the spec'd
    # inputs); fp64 softmax for stability
    q = input.astype(np.float64) @ wq.T.astype(np.float64) + bq
    k = states.astype(np.float64) @ wk.T.astype(np.float64) + bk
    v = states.astype(np.float64) @ wv.T.astype(np.float64) + bv
    scores = np.einsum("bqd,bkd->bqk", q, k) / np.sqrt(float(D))
    scores = np.where(mask == 0, -np.inf, scores)
    m = np.max(scores, axis=2, keepdims=True)
    m = np.where(np.isfinite(m), m, 0.0)
    e = np.exp(scores - m)
    p = e / np.sum(e, axis=2, keepdims=True)
    return np.einsum("bqk,bkd->bqd", p, v).astype(np.float32)


def kernel(input, states, mask, wq, bq, wk, bk, wv, bv):
    input = np.asarray(input, dtype=np.float32)
    states = np.asarray(states, dtype=np.float32)
    mask = np.asarray(mask)
    wq = np.asarray(wq, dtype=np.float32)
    bq = np.asarray(bq, dtype=np.float32)
    wk = np.asarray(wk, dtype=np.float32)
    bk = np.asarray(bk, dtype=np.float32)
    wv = np.asarray(wv, dtype=np.float32)
    bv = np.asarray(bv, dtype=np.float32)
    if np.all(mask != 0):
        return _run_fast(input, states, wq, bq, wk, bk, wv, bv)
    return _numpy_ref(input, states, mask, wq, bq, wk, bk, wv, bv)


# revision 52
# speedup vs baseline: 1.0113x; 1.0113x over previous
"""Trainium2 Bass kernel for nn_Attention (B=8, L=2048, D=512).

Strategy: data-parallel over batch — one batch element per NeuronCore
(8 cores). The host feeds each core its batch slice transposed and
pre-cast to bf16, plus weight-only precomputations (layout/weight
prep; all activation arithmetic runs on device):
  - softmax is shift-invariant, so q.k = (x wq^T + bq).(s wk^T + bk)
    reduces to x A s^T + sw[k] with A = wq^T wk and sw = s.(bq wk)/sqrt(D)
    — the query-constant terms drop. This removes one of the two score
    projections entirely.
Per core:
  - x^T/s^T arrive pre-cast to bf16 in matmul-ready SBUF layouts via
    direct DMA — no on-device casts.  DMA pieces are sized/ordered so
    descriptor rows are large (4KB, 64B-aligned) and land in the V
    phase's consumption order on one HW queue
  - junk warm-up matmuls on memset constants run during the initial
    DMA window so the PE HAM clock-gate is already at 8/8 when real
    matmuls start
  - V projection: moving operand is [wvec | wv^T] (513 cols, split
    257+256 across two PSUM banks), so the per-key score bias
    sw = s.wvec falls out of column 0 of the same matmul group
  - T^T = A^T x^T, one 512-column block per q-block round, interleaved
    with the attention phases so each x block's DMA deadline is a full
    round later
  - scores^T = s^T-stationary x T^T-moving  => [k, q] layout, so the
    softmax key-dim lands on partitions
  - E = exp(scale * scores^T + sw[k]) on ScalarE, sw as the
    per-partition activation bias (no max-subtraction needed:
    shift-invariance again, and scores are O(1) here)
  - context: moving operand is [1 | V] (513 cols, split 257+256), so
    the key-dim softmax denominator is column 0 of the same PSUM
    accumulation — no transposes, row-sum matmuls or DVE adds
  - out = ctx * recip(sum) + bv via fused DVE scalar_tensor_tensor
    (v-bias folds out of the projection: softmax rows sum to 1), in
    two 256-column halves so the output DMA overlaps the second half
All matmuls run in bf16 with fp32 PSUM accumulation.

The mask input is all-ones per the problem spec; kernel() verifies that
on the host and falls back to an exact numpy implementation for any
other mask. A per-batch spot-check guards the device path (retry, then
exact-host fallback) so out-of-spec inputs or a bad run can never
return wrong results.
"""

import numpy as np

B, L, D = 8, 2048, 512
P = 128
LT = L // P  # 16 l-tiles
DC = D // P  # 4 d/e chunks
NQ = 512  # q-block width
QB = L // NQ  # 4 q blocks
NB = L // NQ  # 4 l-blocks (512 rows each)
N_CORES = 8
SCALE = 1.0 / float(np.sqrt(D))
N_WARM = 48  # junk warm-up matmuls (N=128 each) during the DMA head

_cache = {}


def _build_fast():
    import concourse.tile as tile
    from concourse import bacc, mybir
    from concourse.bass import ds

    F32 = mybir.dt.float32
    BF16 = mybir.dt.bfloat16
    AF = mybir.ActivationFunctionType

    nc = bacc.Bacc(
        "TRN2", target_bir_lowering=False, debug=False, num_devices=N_CORES
    )
    # all big tensors arrive pre-transposed/pre-cast in SBUF layout,
    # arranged for large contiguous DMA descriptor rows (4 KB where
    # possible — small rows run the DGE at ~70 GB/s instead of ~300)
    xT_ext = nc.dram_tensor(
        "inputT", [P, NB, DC, NQ], BF16, kind="ExternalInput"
    )
    # head = [wvA | pad | s-tile0 | s-tile1]: wvA = [wvec | wv^T] per
    # chunk (col 0 = (bq @ wk) * scale), padded to a 64B-aligned row
    # size — misaligned DMA rows run at roughly half bandwidth
    NWVA_PAD = DC * (D + 1) + 28  # 2080 cols = 4160 B, 64-aligned
    NHEAD = NWVA_PAD + 2 * D
    head_ext = nc.dram_tensor("headW", [P, NHEAD], BF16, kind="ExternalInput")
    # states tiles 2-15 partition-major: 4 KB rows per piece
    sb_ext = nc.dram_tensor(
        "statesB", [P, LT - 2, DC * P], BF16, kind="ExternalInput"
    )
    # amat = wq.T @ wk (scores reduce to x @ amat @ s.T plus a per-key
    # bias; the query-constant terms drop out of the softmax)
    amat_ext = nc.dram_tensor("amat", [P, DC * D], BF16, kind="ExternalInput")
    bv_ext = nc.dram_tensor("bv", [D], F32, kind="ExternalInput")
    out_ext = nc.dram_tensor("out", [L, D], F32, kind="ExternalOutput")

    with tile.TileContext(nc) as tc:
        with (
            tc.tile_pool(name="consts", bufs=1) as consts,
            tc.tile_pool(name="persist", bufs=1) as persist,
            tc.tile_pool(name="et", bufs=2) as et_pool,
            tc.tile_pool(name="outp", bufs=3) as outp,
            tc.tile_pool(name="psum_mm", bufs=4, space="PSUM") as psum_mm,
            tc.tile_pool(name="psum_ua", bufs=2, space="PSUM") as psum_ua,
            tc.tile_pool(name="psum_ub", bufs=2, space="PSUM") as psum_ub,
        ):
            # constants (no DMA deps — memsets run right after preamble)
            warm_src = consts.tile([P, P], BF16, tag="warm_src")
            nc.gpsimd.memset(warm_src[:], 1.0)
            ones_st = consts.tile([1, P], BF16, tag="ones_st")
            nc.gpsimd.memset(ones_st[:], 1.0)

            # persistent bf16 tensors
            xT = persist.tile([P, NB, DC, NQ], BF16, tag="xT")
            headw = persist.tile([P, NHEAD], BF16, tag="headw")
            sT = persist.tile([P, LT - 2, DC, P], BF16, tag="sT")
            TT = persist.tile([P, DC, L], BF16, tag="TT")

            # stationary s-tile / wvA slices (head tiles live in headw)
            def s_stat(t, c):
                if t < 2:
                    return headw[:, ds(NWVA_PAD + (t * DC + c) * P, P)]
                return sT[:, t - 2, c, :]

            def wvA_sl(c, off, n):
                return headw[:, ds(c * (D + 1) + off, n)]
            # V_aug = [1 | V]: col 0 is all-ones so the context matmul's
            # first PSUM column accumulates the softmax denominator
            V_aug = persist.tile([P, LT, D + 1], BF16, tag="V_aug")
            nc.gpsimd.memset(V_aug[:, :, ds(0, 1)], 1.0)
            amat = persist.tile([P, DC, D], BF16, tag="amat")
            sw_sb = persist.tile([P, LT], F32, tag="sw_sb")

            # ---- warm-up matmuls: keep the PE busy through the DMA
            # head so HAM un-throttles before real work arrives ----
            # rides a context-pool bank: the first context matmul is
            # ~30us after the last warm-up, so the WAW dep is free
            warm_ps = psum_ua.tile([P, NQ], F32, tag="ps_ua")
            for _ in range(N_WARM):
                nc.tensor.matmul(
                    warm_ps[:, ds(0, P)],
                    warm_src[:],
                    warm_src[:],
                    start=True,
                    stop=True,
                )

            # ---- DMA issue order = landing order (one FIFO): the
            # combined head first, then s pieces at the V phase's
            # consumption cadence, then weights and x blocks ----
            nc.sync.dma_start(
                headw[:, ds(0, NWVA_PAD)], head_ext.ap()[:, ds(0, NWVA_PAD)]
            )
            nc.sync.dma_start(
                headw[:, ds(NWVA_PAD, 2 * D)],
                head_ext.ap()[:, ds(NWVA_PAD, 2 * D)],
            )
            sb_ap = sb_ext.ap().rearrange("p t (c j) -> p t c j", c=DC)
            for r0, nt in ((0, 2), (2, 2), (4, 4), (8, 4), (12, 2)):
                nc.sync.dma_start(
                    sT[:, ds(r0, nt), :, :], sb_ap[:, ds(r0, nt), :, :]
                )
            bv_f32 = consts.tile([1, D], F32, tag="bv_f32")
            nc.sync.dma_start(
                bv_f32[:], bv_ext.ap().rearrange("(one d) -> one d", one=1)
            )
            nc.sync.dma_start(
                amat[:], amat_ext.ap().rearrange("p (c e) -> p c e", c=DC)
            )
            for lb in range(NB):
                nc.sync.dma_start(
                    xT[:, ds(lb, 1), :, :], xT_ext.ap()[:, ds(lb, 1), :, :]
                )

            # ---- Phase A: V projection (+ sw in column 0) ----
            for t in range(LT):
                psA = psum_mm.tile([P, NQ], F32, tag="ps_mm")
                psB = psum_mm.tile([P, NQ], F32, tag="ps_mm")
                for c in range(DC):
                    nc.tensor.matmul(
                        psA[:, ds(0, 257)],
                        s_stat(t, c),
                        wvA_sl(c, 0, 257),
                        start=(c == 0),
                        stop=(c == DC - 1),
                    )
                    nc.tensor.matmul(
                        psB[:, ds(0, 256)],
                        s_stat(t, c),
                        wvA_sl(c, 257, 256),
                        start=(c == 0),
                        stop=(c == DC - 1),
                    )
                nc.vector.tensor_copy(sw_sb[:, ds(t, 1)], psA[:, ds(0, 1)])
                nc.vector.tensor_copy(
                    V_aug[:, t, ds(1, 256)], psA[:, ds(1, 256)]
                )
                nc.vector.tensor_copy(
                    V_aug[:, t, ds(257, 256)], psB[:, ds(0, 256)]
                )

            # BV: bv broadcast to all 128 partitions (ones-column matmul);
            # rides the PE stream between phases, consumed only at the end
            bv_bf = consts.tile([1, D], BF16, tag="bv_bf")
            nc.vector.tensor_copy(bv_bf[:], bv_f32[:])
            bv_ps = psum_mm.tile([P, D], F32, tag="ps_mm")
            nc.tensor.matmul(
                bv_ps[:], ones_st[:, :], bv_bf[:, :], start=True, stop=True
            )
            BV = consts.tile([P, D], F32, tag="BV")
            nc.vector.tensor_copy(BV[:], bv_ps[:])

            # ---- Phases B+C interleaved per q-block: project T for
            # block qb (x block qb only), then scores + context.  This
            # defers each x block's deadline by a full qb round, so the
            # DMA link can prioritize the V phase's s tiles. ----
            for qb in range(QB):
                lb = qb
                for e in range(DC):
                    ps = psum_mm.tile([P, NQ], F32, tag="ps_mm")
                    for c in range(DC):
                        nc.tensor.matmul(
                            ps[:],
                            amat[:, c, ds(e * P, P)],
                            xT[:, lb, c, :],
                            start=(c == 0),
                            stop=(c == DC - 1),
                        )
                    nc.scalar.copy(TT[:, e, ds(lb * NQ, NQ)], ps[:])

                ET = et_pool.tile([P, LT, NQ], BF16, tag="ET")
                for kt in range(LT):
                    ps = psum_mm.tile([P, NQ], F32, tag="ps_mm")
                    for e in range(DC):
                        nc.tensor.matmul(
                            ps[:],
                            s_stat(kt, e),
                            TT[:, e, ds(qb * NQ, NQ)],
                            start=(e == 0),
                            stop=(e == DC - 1),
                        )
                    nc.scalar.activation(
                        ET[:, kt, :],
                        ps[:],
                        AF.Exp,
                        bias=sw_sb[:, ds(kt, 1)],
                        scale=SCALE,
                    )

                for j in range(NQ // P):
                    uA = psum_ua.tile([P, NQ], F32, tag="ps_ua")
                    uB = psum_ub.tile([P, NQ], F32, tag="ps_ub")
                    last = qb == QB - 1 and j == NQ // P - 1
                    # final tile: run the full A sweep first so the
                    # first-half store chain overlaps the B sweep and
                    # only STT-B + one store remain on the tail
                    for kt in range(LT):
                        nc.tensor.matmul(
                            uA[:, ds(0, 257)],
                            ET[:, kt, ds(j * P, P)],
                            V_aug[:, kt, ds(0, 257)],
                            start=(kt == 0),
                            stop=(kt == LT - 1),
                        )
                        if not last:
                            nc.tensor.matmul(
                                uB[:, ds(0, 256)],
                                ET[:, kt, ds(j * P, P)],
                                V_aug[:, kt, ds(257, 256)],
                                start=(kt == 0),
                                stop=(kt == LT - 1),
                            )
                    rec = outp.tile([P, 1], F32, tag="rec")
                    nc.vector.reciprocal(rec[:], uA[:, ds(0, 1)])
                    o = outp.tile([P, D], F32, tag="o")
                    row = (qb * (NQ // P) + j) * P
                    nc.vector.scalar_tensor_tensor(
                        o[:, ds(0, 256)],
                        uA[:, ds(1, 256)],
                        rec[:],
                        BV[:, ds(0, 256)],
                        op0=mybir.AluOpType.mult,
                        op1=mybir.AluOpType.add,
                    )
                    nc.sync.dma_start(
                        out_ext.ap()[ds(row, P), ds(0, 256)], o[:, ds(0, 256)]
                    )
                    if last:
                        for kt in range(LT):
                            nc.tensor.matmul(
                                uB[:, ds(0, 256)],
                                ET[:, kt, ds(j * P, P)],
                                V_aug[:, kt, ds(257, 256)],
                                start=(kt == 0),
                                stop=(kt == LT - 1),
                            )
                    nc.vector.scalar_tensor_tensor(
                        o[:, ds(256, 256)],
                        uB[:, ds(0, 256)],
                        rec[:],
                        BV[:, ds(256, 256)],
                        op0=mybir.AluOpType.mult,
                        op1=mybir.AluOpType.add,
                    )
                    # final tile: issue the B-half store from the scalar
                    # queue so the two issues overlap on the tail path
                    last = qb == QB - 1 and j == NQ // P - 1
                    eng_o = nc.scalar if last else nc.sync
                    eng_o.dma_start(
                        out_ext.ap()[ds(row, P), ds(256, 256)],
                        o[:, ds(256, 256)],
                    )

    nc.compile()
    return nc


def _make_in_maps(input, states, wq, bq, wk, bk, wv, bv):
    import ml_dtypes

    BF = ml_dtypes.bfloat16
    wq64 = np.asarray(wq, dtype=np.float64)
    wk64 = np.asarray(wk, dtype=np.float64)
    amat = (wq64.T @ wk64).astype(np.float32)  # [D, D]
    wvec = ((np.asarray(bq, dtype=np.float64) @ wk64) * SCALE).astype(
        np.float32
    )  # [D]
    wvT = np.asarray(wv, dtype=np.float32).T  # [D, D]
    wvA = np.concatenate([wvec[:, None], wvT], axis=1)  # [D, D+1]
    amat_h = np.ascontiguousarray(
        amat.reshape(DC, P, D).transpose(1, 0, 2).reshape(P, DC * D).astype(BF)
    )
    wvA_h = wvA.reshape(DC, P, D + 1).transpose(1, 0, 2).astype(BF)  # [p,c,e]
    bv = np.ascontiguousarray(bv, dtype=np.float32)
    in_maps = []
    for i in range(N_CORES):
        xt = np.asarray(input[i], dtype=np.float32).T.astype(BF)  # [D, L]
        st = np.asarray(states[i], dtype=np.float32).T.astype(BF)
        # x: [p, block, chunk, col-in-block]
        x_h = np.ascontiguousarray(
            xt.reshape(DC, P, NB, NQ).transpose(1, 2, 0, 3)
        )
        # s tile-major [t, p, c, j]
        s_t = st.reshape(DC, P, LT, P).transpose(2, 1, 0, 3)
        # head = [wvA | pad-to-64B | s-tile0 | s-tile1] per partition row
        head = np.concatenate(
            [
                wvA_h.reshape(P, DC * (D + 1)),
                np.zeros((P, 28), dtype=BF),
                s_t[0].reshape(P, D),
                s_t[1].reshape(P, D),
            ],
            axis=1,
        )
        s_blk = np.ascontiguousarray(
            s_t[2:].transpose(1, 0, 2, 3).reshape(P, LT - 2, DC * P)
        )
        in_maps.append(
            {
                "inputT": x_h,
                "headW": np.ascontiguousarray(head),
                "statesB": s_blk,
                "amat": amat_h,
                "bv": bv,
            }
        )
    return in_maps


def _spot_check(out, input, states, wq, bq, wk, bk, wv, bv):
    """Recompute a few query rows per batch on host; True iff they match."""
    rows = [37, 911, 1500, 2047]
    for i in range(N_CORES):
        k = states[i].astype(np.float64) @ wk.T.astype(np.float64) + bk
        v = states[i].astype(np.float64) @ wv.T.astype(np.float64) + bv
        for r in rows:
            q = input[i, r].astype(np.float64) @ wq.T.astype(np.float64) + bq
            s = (k @ q) / np.sqrt(float(D))
            s -= s.max()
            e = np.exp(s)
            ref_row = (e @ v) / e.sum()
            got = out[i, r].astype(np.float64)
            err = np.linalg.norm(got - ref_row) / max(
                np.linalg.norm(ref_row), 1e-30
            )
            if not np.isfinite(err) or err > 0.05:
                return False
    return True


def _run_fast(input, states, wq, bq, wk, bk, wv, bv):
    from concourse.bass_utils import run_bass_kernel_spmd

    if "fast" not in _cache:
        _cache["fast"] = _build_fast()
    nc = _cache["fast"]
    in_maps = _make_in_maps(input, states, wq, bq, wk, bk, wv, bv)
    for _attempt in range(2):
        res = run_bass_kernel_spmd(nc, in_maps, core_ids=list(range(N_CORES)))
        out = np.stack(
            [res.results[i]["out"] for i in range(N_CORES)], axis=0
        )
        if _spot_check(out, input, states, wq, bq, wk, bk, wv, bv):
            return out
    # two bad device runs in a row: fall back to the exact host path
    ones = np.ones((B, L, L), dtype=np.int32)
    return _numpy_ref(input, states, ones, wq, bq, wk, bk, wv, bv)


def _numpy_ref(input, states, mask, wq, bq, wk, bk, wv, bv):
    # exact fallback for non-all-ones masks (never taken for the spec'd
    # inputs); fp64 softmax for stability
    q = input.astype(np.float64) @ wq.T.astype(np.float64) + bq
    k = states.astype(np.float64) @ wk.T.astype(np.float64) + bk
    v = states.astype(np.float64) @ wv.T.astype(np.float64) + bv
    scores = np.einsum("bqd,bkd->bqk", q, k) / np.sqrt(float(D))
    scores = np.where(mask == 0, -np.inf, scores)
    m = np.max(scores, axis=2, keepdims=True)
    m = np.where(np.isfinite(m), m, 0.0)
    e = np.exp(scores - m)
    p = e / np.sum(e, axis=2, keepdims=True)
    return np.einsum("bqk,bkd->bqd", p, v).astype(np.float32)


def kernel(input, states, mask, wq, bq, wk, bk, wv, bv):
    input = np.asarray(input, dtype=np.float32)
    states = np.asarray(states, dtype=np.float32)
    mask = np.asarray(mask)
    wq = np.asarray(wq, dtype=np.float32)
    bq = np.asarray(bq, dtype=np.float32)
    wk = np.asarray(wk, dtype=np.float32)
    bk = np.asarray(bk, dtype=np.float32)
    wv = np.asarray(wv, dtype=np.float32)
    bv = np.asarray(bv, dtype=np.float32)
    if np.all(mask != 0):
        return _run_fast(input, states, wq, bq, wk, bk, wv, bv)
    return _numpy_ref(input, states, mask, wq, bq, wk, bk, wv, bv)
